# revision 1
# baseline (speedup 1.0000x reference)
"""Differentiable risk budgeting solve on 8 Trainium2 NeuronCores.

Problem: 20 unrolled iterations of
    Sw   = einsum('bij,bj->bi', sigma, w)
    grad = 2*Sw - beta + lam_s*sign(w) + 2*lam_t*(w - w_prev)
    w    = proj(w - 0.05*grad)          # clip/renorm twice
with B=32768, P=45.

Strategy: pure data parallel over 8 cores (4096 batch rows each).
sigma is cast to fp16 on the host and kept entirely SBUF-resident
(~127KiB/partition), so HBM traffic is one half-precision pass.

Per iteration the batched matvec runs on the VectorEngine as an fp16
elementwise multiply (2x DVE perf mode for packed 2-byte operands)
followed by an in-place pairwise tree reduction over the contraction
axis (45->23->12->6->3->2->1) of fp16 tensor_tensor adds -- measured
~3x faster than the mode-less tensor_reduce.  The serial
update/projection chain runs entirely on the DVE with in-place clips
(cross-engine hops stall it); GPSIMD computes the off-critical-path
prep term D - s*sign(w) + cw*w, emitted BEFORE the bulk so it hides
under the multiply (its ISA accepts only tensor_tensor add/sub/mult
and tensor_copy).  The Scalar engine is used solely as the DMA
dispatch queue: dependency-chained Act compute ops measured ~6us
each.  Tiles are processed in PAIRS (bulk per 512-row half, chain
once per 1024-row pair) and the 20 iterations emit ITERATION-OUTER
round-robin across all four pairs, with the projection chain
STAGE-ZIPPED across pairs (all clips, then all sums, then all
reciprocals, ...): the DVE sequencer issues in program order, so
round-robin places ~3 pairs of independent bulk between each pair's
chain and its next iteration, and stage-zipping gives consecutive
chain instructions dependency distance 4 so write-latency stalls
pipeline away -- together measured ~9% faster than letting the
cost-model scheduler order a pair-at-a-time stream.

Update folded to  u = cw*w - 0.1*Sw - s*sign(w) + D  with
cw = 1-0.1*lam_t, s = 0.05*lam_s, D = 0.05*beta + 0.1*lam_t*w_prev
(host-folded, lambdas baked as immediates), sign(w>=0) realized
branch-free as min(w16*6e4, s).  sigma stays UNSCALED in fp16
(products sigma*w ~1e-3 stay in fp16 normal range; pre-scaling by
-0.1 would push them toward subnormals); -0.1 folds into the DVE
scalar_tensor_tensor that adds the tree result, cw multiplies via a
[128,1] constant tile broadcast on GPSIMD.  The reference's +eps
inside renorm shifts results by ~1e-10 relative and is dropped (the
clipped sum is bounded away from zero).
"""

import os
import sys

sys.path.insert(0, "/opt/trn_rl_repo")

import numpy as np

import concourse.bacc as bacc
import concourse.bass as bass
import concourse.mybir as mybir
import concourse.tile as tile
from concourse.bass_utils import run_bass_kernel_spmd

N_CORES = 8
B_TOTAL = 32768
P = 45
BC = B_TOTAL // N_CORES  # 4096 batch rows per core

N_ITER = 20
STEP = 0.05
MAXW = 0.15
EPS = 1e-8
BIGH = 60000.0  # fp16-safe "big": min normal w16 * 6e4 >> s, and 0*6e4 = 0

NB = 4  # batch groups per tile (free dim)
TB = 128 * NB  # batch rows per tile
NT = BC // TB  # tiles per core

# trailing i-rows of the matvec (multiply + tree) offloaded to GPSIMD,
# which otherwise idles while the DVE does all the bulk work
GPS_I = int(os.environ.get("RISK_GPS_I", "0"))

F32 = mybir.dt.float32
F16 = mybir.dt.float16
ALU = mybir.AluOpType
AX = mybir.AxisListType


def _tree_steps(n):
    """In-place pairwise halving: a[0:h] += a[n-h:n]; n -> n-h."""
    steps = []
    while n > 1:
        h = n // 2
        steps.append((h, n))
        n -= h
    return steps  # n=45: [(22,45),(11,23),(6,12),(3,6),(1,3),(1,2)]


def _build_program(cw: float, s: float):
    """Trace the per-core Bass program. cw/s are baked as immediates."""
    c0 = float(np.float32(cw) / np.float32(P) - np.float32(s))
    nc = bacc.Bacc("TRN2", target_bir_lowering=False, debug=False)

    sig_d = nc.dram_tensor("sigma16", [BC, P, P], F16, kind="ExternalInput").ap()
    d_d = nc.dram_tensor("dvec", [BC, P], F32, kind="ExternalInput").ap()
    w_d = nc.dram_tensor("wout", [BC, P], F32, kind="ExternalOutput").ap()

    reps = int(os.environ.get("RISK_KERNEL_BENCH_REPS", "1"))

    import contextlib

    steps = _tree_steps(P)

    with tile.TileContext(nc) as tc:
        with (
            tc.tile_pool(name="sig", bufs=1) as psig,
            tc.tile_pool(name="prod", bufs=3) as pprod,
            tc.tile_pool(name="wrk", bufs=1) as pwrk,
            tc.For_i(0, reps, 1) if reps > 1 else contextlib.nullcontext(),
        ):
            # broadcastable [128,1] constants (free-dim broadcast only)
            c_cw = psig.tile([128, 1], F32, tag="c_cw")
            nc.gpsimd.memset(c_cw[:], cw)
            c_m01 = psig.tile([128, 1], F32, tag="c_m01")
            nc.gpsimd.memset(c_m01[:], -2.0 * STEP)
            c_m01p = psig.tile([128, 1], F32, tag="c_m01p")
            nc.gpsimd.memset(c_m01p[:], -2.0 * STEP / P)

            def bc3(t):  # [128,1] -> [128, NB, P] free broadcast
                return t[:].unsqueeze(2).broadcast_to([128, NB, P])

            # ---- resident sigma fp16 + D tiles ----
            sigs, dts = [], []
            for t in range(NT):
                base = t * TB
                sig = psig.tile([128, NB * P * P], F16, tag=f"sig{t}")
                sig4 = sig[:].rearrange("p (g i j) -> p g i j", g=NB, i=P)
                for g in range(NB):
                    nc.scalar.dma_start(
                        sig4[:, g], sig_d[base + g * 128 : base + (g + 1) * 128]
                    )
                dt_ = psig.tile([128, NB * P], F32, tag=f"d{t}")
                dt3 = dt_[:].rearrange("p (g j) -> p g j", g=NB)
                for g in range(NB):
                    nc.scalar.dma_start(
                        dt3[:, g], d_d[base + g * 128 : base + (g + 1) * 128]
                    )
                sigs.append((sig, sig4))
                dts.append((dt_, dt3))

            # ---- paired tiles, ITERATION-OUTER round-robin emission:
            # the DVE sequencer issues in program order, so emission order
            # is execution order.  Emitting iteration it for ALL pairs
            # before iteration it+1 puts ~3 pairs of independent bulk work
            # between every dependent chain, hiding its latency without
            # relying on the cost-model scheduler's (underestimated)
            # latency guesses.  Chain ops run once per 1024-row pair with
            # in-place clips; prep on GPSIMD overlaps the multiplies. ----
            NBP = 2 * NB
            NP2 = NT // 2
            states = []
            for pt in range(NP2):
                w32 = pwrk.tile([128, NBP * P], F32, tag=f"w32_{pt}")
                e1 = pwrk.tile([128, NBP * P], F32, tag=f"e1_{pt}")
                e2 = pwrk.tile([128, NBP * P], F32, tag=f"e2_{pt}")
                r = pwrk.tile([128, NBP], F32, tag=f"r_{pt}")
                rr = pwrk.tile([128, NBP], F32, tag=f"rr_{pt}")
                w16 = pwrk.tile([128, NBP * P], F16, tag=f"w16_{pt}")
                u16 = pwrk.tile([128, NBP * P], F16, tag=f"u16_{pt}")
                states.append((w32, e1, e2, r, rr, w16, u16))

            cwb_p = c_cw[:].unsqueeze(2).broadcast_to([128, NBP, P])

            for it in range(N_ITER):
                # --- per-pair: w16 cast, GPSIMD prep, bulk, tree fold ---
                # (prod lifetime stays per-pair so 3 bufs suffice)
                for pt in range(NP2):
                    ta, tb = 2 * pt, 2 * pt + 1
                    w32, e1, e2, r, rr, w16, u16 = states[pt]
                    w32_3 = w32[:].rearrange("p (g j) -> p g j", g=NBP)
                    e1_3 = e1[:].rearrange("p (g j) -> p g j", g=NBP)
                    e2_3 = e2[:].rearrange("p (g j) -> p g j", g=NBP)

                    if it > 0:
                        nc.vector.tensor_copy(w16[:], w32[:])
                        # prep e1 = D - s*sign(w) + cw*w: the GPSIMD ops run
                        # under the bulk below, so the stt never waits
                        nc.vector.tensor_scalar(
                            e2[:], w16[:], BIGH, s, ALU.mult, ALU.min
                        )
                        for hi, t in enumerate((ta, tb)):
                            nc.gpsimd.tensor_tensor(
                                e1_3[:, hi * NB : (hi + 1) * NB],
                                dts[t][1],
                                e2_3[:, hi * NB : (hi + 1) * NB],
                                ALU.subtract,
                            )
                        nc.gpsimd.tensor_tensor(e2_3, w32_3, cwb_p, ALU.mult)
                        nc.gpsimd.tensor_tensor(e1[:], e1[:], e2[:], ALU.add)

                    prod4s = []
                    for hi, t in enumerate((ta, tb)):
                        sig4 = sigs[t][1]
                        prod = pprod.tile([128, NB * P * P], F16, tag="prod")
                        prod4 = prod[:].rearrange(
                            "p (g i j) -> p g i j", g=NB, i=P
                        )
                        prod4s.append((prod4, sig4))
                    if it == 0:
                        h, n = steps[0]
                        for prod4, sig4 in prod4s:
                            nc.vector.tensor_tensor(
                                prod4[:, :, :, 0:h],
                                sig4[:, :, :, 0:h],
                                sig4[:, :, :, n - h : n],
                                ALU.add,
                            )
                        for prod4, sig4 in prod4s:
                            nc.vector.tensor_copy(
                                prod4[:, :, :, h : n - h],
                                sig4[:, :, :, h : n - h],
                            )
                        rest = steps[1:]
                    else:
                        for hi, (prod4, sig4) in enumerate(prod4s):
                            w16h = (
                                w16[:]
                                .rearrange("p (g j) -> p g j", g=NBP)[
                                    :, hi * NB : (hi + 1) * NB
                                ]
                                .unsqueeze(2)
                                .broadcast_to([128, NB, P, P])
                            )
                            nc.vector.tensor_tensor(
                                prod4, sig4, w16h, ALU.mult
                            )
                        rest = steps
                    for h, n in rest:
                        for prod4, sig4 in prod4s:
                            nc.vector.tensor_tensor(
                                prod4[:, :, :, 0:h],
                                prod4[:, :, :, 0:h],
                                prod4[:, :, :, n - h : n],
                                ALU.add,
                            )
                    swps = [prod4[:, :, :, 0] for prod4, _ in prod4s]

                    u16_3 = u16[:].rearrange("p (g j) -> p g j", g=NBP)
                    if it == 0:
                        for hi, t in enumerate((ta, tb)):
                            nc.vector.scalar_tensor_tensor(
                                u16_3[:, hi * NB : (hi + 1) * NB],
                                swps[hi],
                                -2.0 * STEP / P,
                                dts[t][1],
                                ALU.mult,
                                ALU.add,
                            )
                        nc.vector.tensor_scalar(u16[:], u16[:], c0, None, ALU.add)
                    else:
                        for hi in range(2):
                            nc.vector.scalar_tensor_tensor(
                                u16_3[:, hi * NB : (hi + 1) * NB],
                                swps[hi],
                                -2.0 * STEP,
                                e1_3[:, hi * NB : (hi + 1) * NB],
                                ALU.mult,
                                ALU.add,
                            )

                # --- projection, STAGE-ZIPPED across pairs: consecutive
                # DVE instructions are independent (dependency distance 4),
                # so chain write-latency stalls pipeline away ---
                def S(pt):
                    w32, e1, e2, r, rr, w16, u16 = states[pt]
                    return (
                        w32[:].rearrange("p (g j) -> p g j", g=NBP),
                        u16,
                        u16[:].rearrange("p (g j) -> p g j", g=NBP),
                        r,
                        rr,
                        rr[:].unsqueeze(2).broadcast_to([128, NBP, P]),
                    )

                for pt in range(NP2):
                    _, u16, _, _, _, _ = S(pt)
                    nc.vector.tensor_scalar(
                        u16[:], u16[:], 0.0, MAXW, ALU.max, ALU.min
                    )
                for pt in range(NP2):
                    _, _, u16_3, r, _, _ = S(pt)
                    nc.vector.tensor_reduce(r[:], u16_3, AX.X, ALU.add)
                for pt in range(NP2):
                    _, _, _, r, rr, _ = S(pt)
                    nc.vector.reciprocal(rr[:], r[:])
                for pt in range(NP2):
                    _, _, u16_3, _, _, rr_b = S(pt)
                    nc.vector.tensor_tensor(u16_3, u16_3, rr_b, ALU.mult)
                for pt in range(NP2):
                    _, u16, _, _, _, _ = S(pt)
                    nc.vector.tensor_scalar(
                        u16[:], u16[:], 0.0, MAXW, ALU.max, ALU.min
                    )
                for pt in range(NP2):
                    _, _, u16_3, r, _, _ = S(pt)
                    nc.vector.tensor_reduce(r[:], u16_3, AX.X, ALU.add)
                for pt in range(NP2):
                    _, _, _, r, rr, _ = S(pt)
                    nc.vector.reciprocal(rr[:], r[:])
                for pt in range(NP2):
                    w32_3, _, u16_3, _, _, rr_b = S(pt)
                    nc.vector.tensor_tensor(w32_3, u16_3, rr_b, ALU.mult)

            # ---- store ----
            for pt in range(NP2):
                w32_3 = states[pt][0][:].rearrange("p (g j) -> p g j", g=NBP)
                for g in range(NBP):
                    nc.scalar.dma_start(
                        w_d[pt * 2 * TB + g * 128 : pt * 2 * TB + (g + 1) * 128],
                        w32_3[:, g],
                    )

    nc.compile()
    return nc


def _fold(beta, w_prev, log_lambda_sparse, log_lambda_turnover):
    lam_s = np.exp(np.float32(log_lambda_sparse), dtype=np.float32)
    lam_t = np.exp(np.float32(log_lambda_turnover), dtype=np.float32)
    cw = float(np.float32(1.0) - np.float32(2 * STEP) * lam_t)
    s = float(np.float32(STEP) * lam_s)
    dvec = (
        np.float32(STEP) * beta + np.float32(2 * STEP) * lam_t * w_prev
    ).astype(np.float32)
    return cw, s, dvec


def make_in_maps(sigma, beta, w_prev, log_lambda_sparse, log_lambda_turnover):
    cw, s, dvec = _fold(beta, w_prev, log_lambda_sparse, log_lambda_turnover)
    sig16 = np.ascontiguousarray(sigma, dtype=np.float32).astype(np.float16)
    in_maps = []
    for c in range(N_CORES):
        sl = slice(c * BC, (c + 1) * BC)
        in_maps.append({"sigma16": sig16[sl], "dvec": dvec[sl]})
    return cw, s, in_maps


def kernel(sigma, beta, w_prev, log_lambda_sparse, log_lambda_turnover):
    beta = np.asarray(beta, dtype=np.float32)
    w_prev = np.asarray(w_prev, dtype=np.float32)
    cw, s, in_maps = make_in_maps(
        sigma, beta, w_prev, log_lambda_sparse, log_lambda_turnover
    )
    nc = _build_program(cw, s)
    res = run_bass_kernel_spmd(nc, in_maps, core_ids=list(range(N_CORES)))
    out = np.concatenate([res.results[c]["wout"] for c in range(N_CORES)], axis=0)
    return out.astype(np.float32)


if __name__ == "__main__":
    rng = np.random.default_rng(0)
    A = rng.standard_normal((B_TOTAL, P, P), dtype=np.float32) * 0.1
    sig = np.einsum("bij,bkj->bik", A, A) + 0.1 * np.eye(P, dtype=np.float32)
    bet = rng.random((B_TOTAL, P), dtype=np.float32)
    bet /= bet.sum(-1, keepdims=True)
    wp = np.full((B_TOTAL, P), 1.0 / P, dtype=np.float32)
    out = kernel(
        sigma=sig,
        beta=bet,
        w_prev=wp,
        log_lambda_sparse=np.float32(-3.0),
        log_lambda_turnover=np.float32(-2.0),
    )
    print(out.shape, out.dtype, out[:2, :5])



# revision 2
# speedup vs baseline: 1.6562x; 1.6562x over previous
"""Differentiable risk budgeting solve on 8 Trainium2 NeuronCores.

Problem: 20 unrolled iterations of
    Sw   = einsum('bij,bj->bi', sigma, w)
    grad = 2*Sw - beta + lam_s*sign(w) + 2*lam_t*(w - w_prev)
    w    = proj(w - 0.05*grad)          # clip/renorm twice
with B=32768, P=45.

Strategy: pure data parallel over 8 cores (4096 batch rows each).
sigma is cast to fp16 on the host, zero-padded to [45,46] rows, and
kept entirely SBUF-resident, so HBM traffic is one half-precision pass.

The batched matvec runs as ONE custom DVE instruction per 128-row tile:
a fused multiply + inclusive prefix-sum (scan) over the whole 45x46
tile stream, with the output access pattern's innermost (j) stride set
to 0 so each row's final running sum lands in that row's slot of a
per-tile cumsum buffer (last-write-wins).  Row dot products are then
recovered with one batched subtract of adjacent cumsum slots.  This
replaces the multiply + 6-step tree reduction of the previous version
(2 passes over sigma per iteration) with a single 1x-mode pass, and
accumulates in fp32 inside the DVE pipe (better precision than the
fp16 tree).  The zero-pad column (sigma[:,:,45]=0) makes products in
the pad slot 0 regardless of w's pad values.

The update/projection chain runs once per iteration at full [128,1440]
width (batched across all 32 tiles, vs per-pair in the old version).
GPSIMD computes the off-critical-path prep term e1 = D - s*sign(w) +
cw*w, emitted BEFORE the bulk so it hides under the scan-dots.

Update folded to  u = cw*w - 0.1*Sw - s*sign(w) + D  with
cw = 1-0.1*lam_t, s = 0.05*lam_s, D = 0.05*beta + 0.1*lam_t*w_prev
(host-folded, lambdas baked as immediates), sign(w>=0) realized
branch-free as min(w*6e4, s).  The reference's +eps inside renorm
shifts results by ~1e-10 relative and is dropped.
"""

import os
import sys

sys.path.insert(0, "/opt/trn_rl_repo")

import numpy as np

import concourse.bacc as bacc
import concourse.bass as bass
import concourse.mybir as mybir
import concourse.tile as tile
from concourse.bass_utils import run_bass_kernel_spmd

N_CORES = 8
B_TOTAL = 32768
P = 45
PJ = 46  # padded row length (even, 4B-aligned rows for fp16)
BC = B_TOTAL // N_CORES  # 4096 batch rows per core
NT = BC // 128  # 32 tiles of 128 rows per core

N_ITER = 20
STEP = 0.05
MAXW = 0.15
BIGH = 60000.0  # min positive fp32 w * 6e4 vs s: sign(w>0) saturates to s

F32 = mybir.dt.float32
F16 = mybir.dt.float16
ALU = mybir.AluOpType
AX = mybir.AxisListType

_SCAN_DOT = None


def _register_scan_dot():
    """Register the fused multiply+scan custom DVE op at runtime."""
    global _SCAN_DOT
    if _SCAN_DOT is not None:
        return _SCAN_DOT
    import concourse.dve_ops as dve_ops_mod
    from concourse.dve_ops import DveOp, OPS
    from concourse.dve_spec import Spec, Src0, Src1, scan, AluOp as SAluOp

    name = "RISK_SCAN_DOT"
    for op in OPS:
        if op.name == name:
            _SCAN_DOT = op
            return op

    def _ref(in0, in1, c0, c1, c2):
        pr = (in0.astype(np.float32) * in1.astype(np.float32)).reshape(
            in0.shape[0], -1
        )
        return np.cumsum(pr, axis=1, dtype=np.float32).reshape(in0.shape)

    sp = Spec(body=scan(SAluOp.ADD, Src0 * Src1), reference=_ref)
    op = DveOp(
        name,
        sp,
        subdim=False,
        uops_sha={"v3": "b3fc3e78a862b7eb", "v4": "bc6a002865d48b97"},
    )
    OPS.append(op)
    dve_ops_mod._SUB_OPCODE_FOR_NAME[name] = (
        dve_ops_mod._CUSTOM_DVE_ROW_BASE + len(OPS) - 1
    )
    dve_ops_mod.CUSTOM_DVE_SPECS[name] = sp
    _SCAN_DOT = op
    return op


def _build_program(cw: float, s: float):
    """Trace the per-core Bass program. cw/s are baked as immediates."""
    scan_dot = _register_scan_dot()
    nc = bacc.Bacc("TRN2", target_bir_lowering=False, debug=False)

    sig_d = nc.dram_tensor("sigma16", [BC, P * PJ], F16, kind="ExternalInput").ap()
    d_d = nc.dram_tensor("dvec", [BC, P], F32, kind="ExternalInput").ap()
    w_d = nc.dram_tensor("wout", [BC, P], F32, kind="ExternalOutput").ap()

    reps = int(os.environ.get("RISK_KERNEL_BENCH_REPS", "1"))

    import contextlib

    with tile.TileContext(nc) as tc:
        with (
            tc.tile_pool(name="sig", bufs=1) as psig,
            tc.tile_pool(name="wrk", bufs=1) as pwrk,
        ):
            # [128,1] constant for free-dim broadcast on GPSIMD
            c_cw = pwrk.tile([128, 1], F32, tag="c_cw")
            nc.gpsimd.memset(c_cw[:], cw)

            # ---- resident sigma fp16 (padded rows) + D ----
            sigs = []
            for t in range(NT):
                sig = psig.tile([128, P * PJ], F16, tag=f"sig{t}")
                nc.scalar.dma_start(sig[:], sig_d[t * 128 : (t + 1) * 128])
                sigs.append(sig)
            dt_ = pwrk.tile([128, NT * P], F32, tag="dvec")
            dt3 = dt_[:].rearrange("p (t j) -> p t j", t=NT)
            for t in range(NT):
                nc.scalar.dma_start(dt3[:, t], d_d[t * 128 : (t + 1) * 128])

            # ---- state ----
            w32p = pwrk.tile([128, NT * PJ], F32, tag="w32p")  # padded w
            nc.gpsimd.memset(w32p[:], 1.0 / P)
            cum = pwrk.tile([128, NT * PJ], F32, tag="cum")
            nc.gpsimd.memset(cum[:], 0.0)  # slot t*PJ+0 stays 0 forever
            e1 = pwrk.tile([128, NT * P], F32, tag="e1")
            e2 = pwrk.tile([128, NT * P], F32, tag="e2")
            u = pwrk.tile([128, NT * P], F32, tag="u")
            r = pwrk.tile([128, NT], F32, tag="r")
            rr = pwrk.tile([128, NT], F32, tag="rr")

            w32p3 = w32p[:].rearrange("p (t j) -> p t j", j=PJ)  # [128,NT,46]
            w32s = w32p3[:, :, 0:P]  # [128,NT,45] strided view
            cum3 = cum[:].rearrange("p (t j) -> p t j", j=PJ)
            cum_hi = cum3[:, :, 1 : 1 + P]  # [128,NT,45]
            cum_lo = cum3[:, :, 0:P]
            e1_3 = e1[:].rearrange("p (t j) -> p t j", t=NT)
            e2_3 = e2[:].rearrange("p (t j) -> p t j", t=NT)
            u3 = u[:].rearrange("p (t j) -> p t j", t=NT)
            rr_b = rr[:].unsqueeze(2).broadcast_to([128, NT, P])
            cwb = c_cw[:].unsqueeze(2).broadcast_to([128, NT, P])

            with tc.For_i(0, reps, 1) if reps > 1 else contextlib.nullcontext():
                for it in range(N_ITER):
                    # prep e1 = D - s*sign(w) + cw*w on GPSIMD (hides under bulk)
                    nc.vector.tensor_scalar(
                        e2_3, w32s, BIGH, s, ALU.mult, ALU.min
                    )
                    nc.gpsimd.tensor_tensor(e1_3, dt3, e2_3, ALU.subtract)
                    nc.gpsimd.tensor_tensor(e2_3, w32s, cwb, ALU.mult)
                    nc.gpsimd.tensor_tensor(e1[:], e1[:], e2[:], ALU.add)

                    # bulk: fused multiply+scan per tile; page-end cumsums
                    # land in cum[t*PJ+1 .. t*PJ+45] via stride-0 j writes
                    for t in range(NT):
                        sig3 = sigs[t][:].rearrange("p (i j) -> p i j", i=P)
                        w_b = (
                            w32p3[:, t].unsqueeze(1).broadcast_to([128, P, PJ])
                        )
                        cum_o = (
                            cum3[:, t, 1 : 1 + P]
                            .unsqueeze(2)
                            .broadcast_to([128, P, PJ])
                        )
                        nc.vector._custom_dve(
                            scan_dot, out=cum_o, in0=sig3, in1=w_b
                        )

                    # Sw = cum_hi - cum_lo;  u = e1 - 0.1*Sw
                    nc.vector.tensor_tensor(u3, cum_hi, cum_lo, ALU.subtract)
                    nc.vector.scalar_tensor_tensor(
                        u[:], u[:], -2.0 * STEP, e1[:], ALU.mult, ALU.add
                    )
                    # projection: clip, renorm, clip, renorm
                    nc.vector.tensor_scalar(
                        u[:], u[:], 0.0, MAXW, ALU.max, ALU.min
                    )
                    nc.vector.tensor_reduce(r[:], u3, AX.X, ALU.add)
                    nc.vector.reciprocal(rr[:], r[:])
                    nc.vector.tensor_tensor(u3, u3, rr_b, ALU.mult)
                    nc.vector.tensor_scalar(u[:], u[:], MAXW, None, ALU.min)
                    nc.vector.tensor_reduce(r[:], u3, AX.X, ALU.add)
                    nc.vector.reciprocal(rr[:], r[:])
                    nc.vector.tensor_tensor(w32s, u3, rr_b, ALU.mult)

            # ---- store ----
            for t in range(NT):
                nc.scalar.dma_start(w_d[t * 128 : (t + 1) * 128], w32p3[:, t, 0:P])

    nc.compile()
    return nc


def _fold(beta, w_prev, log_lambda_sparse, log_lambda_turnover):
    lam_s = np.exp(np.float32(log_lambda_sparse), dtype=np.float32)
    lam_t = np.exp(np.float32(log_lambda_turnover), dtype=np.float32)
    cw = float(np.float32(1.0) - np.float32(2 * STEP) * lam_t)
    s = float(np.float32(STEP) * lam_s)
    dvec = (
        np.float32(STEP) * beta + np.float32(2 * STEP) * lam_t * w_prev
    ).astype(np.float32)
    return cw, s, dvec


def make_in_maps(sigma, beta, w_prev, log_lambda_sparse, log_lambda_turnover):
    cw, s, dvec = _fold(beta, w_prev, log_lambda_sparse, log_lambda_turnover)
    sig16p = np.zeros((B_TOTAL, P, PJ), dtype=np.float16)
    sig16p[:, :, :P] = np.asarray(sigma, dtype=np.float32)
    sig16p = sig16p.reshape(B_TOTAL, P * PJ)
    in_maps = []
    for c in range(N_CORES):
        sl = slice(c * BC, (c + 1) * BC)
        in_maps.append({"sigma16": sig16p[sl], "dvec": dvec[sl]})
    return cw, s, in_maps


def kernel(sigma, beta, w_prev, log_lambda_sparse, log_lambda_turnover):
    beta = np.asarray(beta, dtype=np.float32)
    w_prev = np.asarray(w_prev, dtype=np.float32)
    cw, s, in_maps = make_in_maps(
        sigma, beta, w_prev, log_lambda_sparse, log_lambda_turnover
    )
    nc = _build_program(cw, s)
    res = run_bass_kernel_spmd(nc, in_maps, core_ids=list(range(N_CORES)))
    out = np.concatenate([res.results[c]["wout"] for c in range(N_CORES)], axis=0)
    return out.astype(np.float32)


if __name__ == "__main__":
    rng = np.random.default_rng(0)
    A = rng.standard_normal((B_TOTAL, P, P), dtype=np.float32) * 0.1
    sig = np.einsum("bij,bkj->bik", A, A) + 0.1 * np.eye(P, dtype=np.float32)
    bet = rng.random((B_TOTAL, P), dtype=np.float32)
    bet /= bet.sum(-1, keepdims=True)
    wp = np.full((B_TOTAL, P), 1.0 / P, dtype=np.float32)
    out = kernel(
        sigma=sig,
        beta=bet,
        w_prev=wp,
        log_lambda_sparse=np.float32(-3.0),
        log_lambda_turnover=np.float32(-2.0),
    )
    # quick numpy check on first 256 rows
    lam_s = np.exp(np.float32(-3.0))
    lam_t = np.exp(np.float32(-2.0))
    n = 256
    w = np.full((n, P), 1.0 / P, dtype=np.float32)
    for _ in range(N_ITER):
        Sw = np.einsum("bij,bj->bi", sig[:n], w)
        g = 2 * Sw - bet[:n] + lam_s * np.sign(w) + 2 * lam_t * (w - wp[:n])
        w = w - STEP * g
        for _ in range(2):
            w = np.clip(w, 0, MAXW)
            w = w / (w.sum(-1, keepdims=True) + 1e-8)
    err = np.abs(out[:n] - w).max() / np.abs(w).max()
    print(out.shape, out.dtype, "absmax-rel vs numpy (256 rows):", err)


# revision 3
# speedup vs baseline: 2.3702x; 1.4311x over previous
"""Differentiable risk budgeting solve on 8 Trainium2 NeuronCores.

Problem: 20 unrolled iterations of
    Sw   = einsum('bij,bj->bi', sigma, w)
    grad = 2*Sw - beta + lam_s*sign(w) + 2*lam_t*(w - w_prev)
    w    = proj(w - 0.05*grad)          # clip/renorm twice
with B=32768, P=45.

Strategy: pure data parallel over 8 cores (4096 batch rows each).
sigma is cast to fp16 on the host, rows zero-padded 45->46, and kept
entirely SBUF-resident, so HBM traffic is one half-precision pass.

The batched matvec runs as ONE custom DVE instruction per 128-row tile
(RISK_SEG_DOT): a fused multiply + segmented (per-row-reset)
accumulation over the [45,46] tile stream.  The custom op carries a
hand-authored 2x_1P micro-op program (packed fp16 pairs: MUL lo, MUL
hi, pair add, accumulate; accumulator reset at row boundaries via the
SUB_DIM_DONE step state; seed state zeroes the accumulator with
LOGICAL_XOR(acc,acc) so stale pipe state can't leak in) and advertises
perf_max=1, so the DVE streams 2 elements/cycle - measured ~1.75x the
1x fused op and ~2.3x the old multiply+tree.  Accumulation is fp32
inside the pipe; only the written per-pair cums quantize to fp16.
Row dots are the written values at each row's last (zero-pad) column;
GPSIMD extracts them (strided copy) in groups of 4 tiles, overlapped
under the next tiles' bulk, while GPSIMD also computes the
off-critical-path prep term e1 = D - s*sign(w) + cw*w.

The update/projection chain runs once per iteration at full [128,1440]
width in fp32.  Update folded to u = cw*w - 0.1*Sw - s*sign(w) + D
with cw = 1-0.1*lam_t, s = 0.05*lam_s, D = 0.05*beta + 0.1*lam_t*w_prev
(host-folded, lambdas baked as immediates), sign(w>=0) realized
branch-free as min(w*6e4, s).  The reference's +eps inside renorm
shifts results by ~1e-10 relative and is dropped.
"""

import os
import sys

sys.path.insert(0, "/opt/trn_rl_repo")

import numpy as np

import concourse.bacc as bacc
import concourse.bass as bass
import concourse.mybir as mybir
import concourse.tile as tile
from concourse.bass_utils import run_bass_kernel_spmd

N_CORES = 8
B_TOTAL = 32768
P = 45
PJ = 46  # padded row length (even, keeps packed pairs page-aligned)
BC = B_TOTAL // N_CORES  # 4096 batch rows per core
NT = BC // 128  # 32 tiles of 128 rows per core
NSLOT = 4  # rotating full-cum buffers

N_ITER = 20
STEP = 0.05
MAXW = 0.15
BIGH = 60000.0

F32 = mybir.dt.float32
F16 = mybir.dt.float16
ALU = mybir.AluOpType
AX = mybir.AxisListType

# --------------------------------------------------------------------------
# RISK_SEG_DOT: custom DVE op, hand-authored 1x + 2x_1P programs.
# --------------------------------------------------------------------------

_SEG_DOT = None
_PATCHED = False


def _build_uops():
    from concourse.dve_uop import (
        AluInp,
        AluOp as UAlu,
        DelayInp,
        InpSel,
        OutPath,
        OutSel,
        Trigger,
        UopConfig,
        UopDpConfig,
    )

    PD = DelayInp.PREV_DELAY

    def dp(op, a, b, delay_sel=None, delay_en=None):
        d = UopDpConfig(op=op, alu_src0=a, alu_src1=b, alu_out_enable=1)
        if delay_sel is not None:
            d.delay = list(delay_sel)
        if delay_en is not None:
            d.delay_enable = list(delay_en)
        return d

    def tail(dps):
        while len(dps) < 8:
            dps.append(dp(UAlu.BYPASS, AluInp.PREV_ALU_OUT, AluInp.PREV_ALU_OUT))
        return dps

    def uop(inp, inp_en, dps, trigger, next_uop, repeat, req, wr):
        u = UopConfig()
        u.inp = list(inp) + [InpSel.ZERO] * (8 - len(inp))
        u.inp_enable = list(inp_en) + [0] * (8 - len(inp_en))
        u.datapath_config = dps
        u.trigger = tuple(trigger)
        u.next_uop = tuple(next_uop)
        u.repeat_count = repeat
        u.require_inp0, u.require_inp1 = req
        u.out = {p: OutSel.ALU_OUT for p in OutPath}
        u.out_enable = {
            OutPath.WR0_LO: wr[0],
            OutPath.WR0_HI: wr[1],
            OutPath.WR1_LO: 0,
            OutPath.WR1_HI: 0,
        }
        return u

    T = Trigger

    def fsm(inp, inp_en, mk_dp, wr):
        seed = uop(
            inp, inp_en,
            mk_dp(UAlu.LOGICAL_XOR, AluInp.CURR_ALU_OUT, AluInp.CURR_ALU_OUT),
            (T.COUNT, T.NONE, T.NONE), (1, 0, 0), 1, (0, 0), (0, 0),
        )
        steady = uop(
            inp, inp_en,
            mk_dp(UAlu.ADD, AluInp.CURR_ALU_OUT, AluInp.PREV_ALU_OUT),
            (T.SRC_TENSOR_DONE, T.SUB_DIM_DONE, T.NONE), (0, 2, 0), 0,
            (1, 1), wr,
        )
        step = uop(
            inp, inp_en,
            mk_dp(UAlu.BYPASS, AluInp.PREV_ALU_OUT, AluInp.PREV_ALU_OUT),
            (T.SRC_TENSOR_DONE, T.SUB_DIM_DONE, T.COUNT), (0, 2, 1), 1,
            (1, 1), wr,
        )
        return [seed, steady, step]

    # 1x: stage0 product, stage1 accumulator
    inp1 = [InpSel.ZERO, InpSel.SRC_0, InpSel.SRC_1]
    en1 = [0, 1, 1]
    carry = [PD] * 7

    def mk1(acc_op, a, b):
        return tail([
            dp(UAlu.MULTIPLY, AluInp.PREV_DELAY_0, AluInp.PREV_DELAY_1,
               carry, [1, 1, 0, 0, 0, 0, 0]),
            dp(acc_op, a, b, carry, [1, 1, 0, 0, 0, 0, 0]),
        ])

    # 2x: packed pairs; stage0 MUL lo, stage1 MUL hi (capture lo),
    # stage2 pair=lo+hi, stage3 accumulator; acc -> both 16-bit halves
    inp2 = [InpSel.ZERO, InpSel.SRC_0, InpSel.SRC_1,
            InpSel.SRC_0_HI, InpSel.SRC_1_HI]
    en2 = [0, 1, 1, 1, 1]

    def mk2(acc_op, a, b):
        return tail([
            dp(UAlu.MULTIPLY, AluInp.PREV_DELAY_0, AluInp.PREV_DELAY_1,
               carry, [0, 0, 1, 1, 0, 0, 0]),
            dp(UAlu.MULTIPLY, AluInp.PREV_DELAY_2, AluInp.PREV_DELAY_3,
               [DelayInp.PREV_ALU_OUT] + [PD] * 6, [1, 0, 0, 0, 0, 0, 0]),
            dp(UAlu.ADD, AluInp.PREV_ALU_OUT, AluInp.PREV_DELAY_0),
            dp(acc_op, a, b),
        ])

    return fsm(inp1, en1, mk1, (1, 0)), fsm(inp2, en2, mk2, (1, 1))


def _patch_perf_max():
    global _PATCHED
    if _PATCHED:
        return
    import concourse.bass as bass_mod

    isa_mod = bass_mod.bass_isa
    orig = isa_mod.InstCustomDveAnt

    def patched(*args, **kw):
        if kw.get("op_name") == "RISK_SEG_DOT":
            kw.setdefault("perf_max", 1)
        return orig(*args, **kw)

    isa_mod.InstCustomDveAnt = patched
    _PATCHED = True


def _register_seg_dot():
    global _SEG_DOT
    _patch_perf_max()
    if _SEG_DOT is not None:
        return _SEG_DOT
    import concourse.dve_ops as dve_ops_mod
    from concourse.dve_ops import DveOp, OPS, _COMPILE_CACHE
    from concourse.dve_spec import Spec, Src0, Src1, scan, AluOp as SAluOp
    from concourse.dve_uop import DveOpSpec

    name = "RISK_SEG_DOT"
    for op in OPS:
        if op.name == name:
            _SEG_DOT = op
            return op

    def _ref(in0, in1, c0, c1, c2):
        pr = in0.astype(np.float32) * in1.astype(np.float32)
        return np.cumsum(pr, axis=-1, dtype=np.float32)

    sp = Spec(body=scan(SAluOp.ADD, Src0 * Src1), reference=_ref)
    row = dve_ops_mod._CUSTOM_DVE_ROW_BASE + len(OPS)
    uops_1x, uops_2x = _build_uops()
    spec_obj = DveOpSpec(
        name=name, opcode=row, uops=uops_1x, uops_2x=uops_2x,
        perf_max=1, rd1_en=True,
    )
    spec_obj.validate("v3")
    shas = {v: spec_obj.sha(v) for v in ("v3", "v4")}
    op = DveOp(name, sp, subdim=True, uops_sha=shas)
    OPS.append(op)
    dve_ops_mod._SUB_OPCODE_FOR_NAME[name] = row
    dve_ops_mod.CUSTOM_DVE_SPECS[name] = sp
    for v in ("v3", "v4"):
        _COMPILE_CACHE[(name, v)] = spec_obj
    _SEG_DOT = op
    return op


# --------------------------------------------------------------------------
# Kernel
# --------------------------------------------------------------------------


def _build_program(cw: float, s: float):
    """Trace the per-core Bass program. cw/s are baked as immediates."""
    seg_dot = _register_seg_dot()
    nc = bacc.Bacc("TRN2", target_bir_lowering=False, debug=False)

    sig_d = nc.dram_tensor("sigma16", [BC, P * PJ], F16, kind="ExternalInput").ap()
    d_d = nc.dram_tensor("dvec", [BC, P], F32, kind="ExternalInput").ap()
    w_d = nc.dram_tensor("wout", [BC, P], F32, kind="ExternalOutput").ap()

    reps = int(os.environ.get("RISK_KERNEL_BENCH_REPS", "1"))

    import contextlib

    with tile.TileContext(nc) as tc:
        with (
            tc.tile_pool(name="sig", bufs=1) as psig,
            tc.tile_pool(name="wrk", bufs=1) as pwrk,
        ):
            c_cw = pwrk.tile([128, 1], F32, tag="c_cw")
            nc.gpsimd.memset(c_cw[:], cw)

            # ---- resident sigma fp16 (padded rows) + D ----
            sigs = []
            for t in range(NT):
                sig = psig.tile([128, P * PJ], F16, tag=f"sig{t}")
                nc.scalar.dma_start(sig[:], sig_d[t * 128 : (t + 1) * 128])
                sigs.append(sig)
            dt_ = pwrk.tile([128, NT * P], F32, tag="dvec")
            dt3 = dt_[:].rearrange("p (t j) -> p t j", t=NT)
            for t in range(NT):
                nc.scalar.dma_start(dt3[:, t], d_d[t * 128 : (t + 1) * 128])

            # ---- state ----
            w32 = pwrk.tile([128, NT * P], F32, tag="w32")  # compact fp32 w
            nc.gpsimd.memset(w32[:], 1.0 / P)
            w16p = pwrk.tile([128, NT * PJ], F16, tag="w16p")  # padded fp16 w
            nc.gpsimd.memset(w16p[:], 0.0)
            cums = pwrk.tile([128, NSLOT * P * PJ], F16, tag="cums")
            sw16 = pwrk.tile([128, NT * P], F16, tag="sw16")
            e1 = pwrk.tile([128, NT * P], F32, tag="e1")
            e2 = pwrk.tile([128, NT * P], F32, tag="e2")
            u = pwrk.tile([128, NT * P], F32, tag="u")
            r = pwrk.tile([128, NT], F32, tag="r")
            rr = pwrk.tile([128, NT], F32, tag="rr")

            w32_3 = w32[:].rearrange("p (t j) -> p t j", t=NT)
            w16p3 = w16p[:].rearrange("p (t j) -> p t j", j=PJ)
            e1_3 = e1[:].rearrange("p (t j) -> p t j", t=NT)
            e2_3 = e2[:].rearrange("p (t j) -> p t j", t=NT)
            u3 = u[:].rearrange("p (t j) -> p t j", t=NT)
            sw16_3 = sw16[:].rearrange("p (t j) -> p t j", t=NT)
            rr_b = rr[:].unsqueeze(2).broadcast_to([128, NT, P])
            cwb = c_cw[:].unsqueeze(2).broadcast_to([128, NT, P])
            # page-end view of the cum slots: [slot, page] -> elem 45
            GPT = NT // NSLOT  # tile groups per iteration (extractions)
            cums4 = cums[:].rearrange("p (c i j) -> p c i j", c=NSLOT, i=P)
            ends = cums4[:, :, :, P]  # [128, NSLOT, P] strided fp16

            with tc.For_i(0, reps, 1) if reps > 1 else contextlib.nullcontext():
                for it in range(N_ITER):
                    # prep e1 = D - s*sign(w) + cw*w on GPSIMD (under bulk)
                    nc.vector.tensor_scalar(
                        e2[:], w32[:], BIGH, s, ALU.mult, ALU.min
                    )
                    nc.gpsimd.tensor_tensor(e1[:], dt_[:], e2[:], ALU.subtract)
                    nc.gpsimd.tensor_tensor(e2_3, w32_3, cwb, ALU.mult)
                    nc.gpsimd.tensor_tensor(e1[:], e1[:], e2[:], ALU.add)
                    # w16 cast for the fp16 bulk
                    nc.vector.tensor_copy(w16p3[:, :, 0:P], w32_3)

                    # bulk: segmented scan-dot per tile (2x fp16), cum slots
                    # rotate; GPSIMD extracts row dots per group of NSLOT
                    for g in range(GPT):
                        for k in range(NSLOT):
                            t = g * NSLOT + k
                            sig3 = sigs[t][:].rearrange(
                                "p (i j) -> p i j", i=P
                            )
                            w_b = (
                                w16p3[:, t]
                                .unsqueeze(1)
                                .broadcast_to([128, P, PJ])
                            )
                            nc.vector._custom_dve(
                                seg_dot, out=cums4[:, k], in0=sig3, in1=w_b
                            )
                        nc.gpsimd.tensor_copy(
                            sw16_3[:, g * NSLOT : (g + 1) * NSLOT], ends
                        )

                    # u = e1 - 0.1*Sw ; projection (clip, renorm) x2
                    nc.vector.scalar_tensor_tensor(
                        u[:], sw16[:], -2.0 * STEP, e1[:], ALU.mult, ALU.add
                    )
                    nc.vector.tensor_scalar(
                        u[:], u[:], 0.0, MAXW, ALU.max, ALU.min
                    )
                    nc.vector.tensor_reduce(r[:], u3, AX.X, ALU.add)
                    nc.vector.reciprocal(rr[:], r[:])
                    nc.vector.tensor_tensor(u3, u3, rr_b, ALU.mult)
                    nc.vector.tensor_scalar(u[:], u[:], MAXW, None, ALU.min)
                    nc.vector.tensor_reduce(r[:], u3, AX.X, ALU.add)
                    nc.vector.reciprocal(rr[:], r[:])
                    nc.vector.tensor_tensor(w32_3, u3, rr_b, ALU.mult)

            # ---- store ----
            for t in range(NT):
                nc.scalar.dma_start(w_d[t * 128 : (t + 1) * 128], w32_3[:, t])

    nc.compile()
    return nc


def _fold(beta, w_prev, log_lambda_sparse, log_lambda_turnover):
    lam_s = np.exp(np.float32(log_lambda_sparse), dtype=np.float32)
    lam_t = np.exp(np.float32(log_lambda_turnover), dtype=np.float32)
    cw = float(np.float32(1.0) - np.float32(2 * STEP) * lam_t)
    s = float(np.float32(STEP) * lam_s)
    dvec = (
        np.float32(STEP) * beta + np.float32(2 * STEP) * lam_t * w_prev
    ).astype(np.float32)
    return cw, s, dvec


def make_in_maps(sigma, beta, w_prev, log_lambda_sparse, log_lambda_turnover):
    cw, s, dvec = _fold(beta, w_prev, log_lambda_sparse, log_lambda_turnover)
    sig16p = np.zeros((B_TOTAL, P, PJ), dtype=np.float16)
    sig16p[:, :, :P] = np.asarray(sigma, dtype=np.float32)
    sig16p = sig16p.reshape(B_TOTAL, P * PJ)
    in_maps = []
    for c in range(N_CORES):
        sl = slice(c * BC, (c + 1) * BC)
        in_maps.append({"sigma16": sig16p[sl], "dvec": dvec[sl]})
    return cw, s, in_maps


def kernel(sigma, beta, w_prev, log_lambda_sparse, log_lambda_turnover):
    beta = np.asarray(beta, dtype=np.float32)
    w_prev = np.asarray(w_prev, dtype=np.float32)
    cw, s, in_maps = make_in_maps(
        sigma, beta, w_prev, log_lambda_sparse, log_lambda_turnover
    )
    nc = _build_program(cw, s)
    res = run_bass_kernel_spmd(nc, in_maps, core_ids=list(range(N_CORES)))
    out = np.concatenate([res.results[c]["wout"] for c in range(N_CORES)], axis=0)
    return out.astype(np.float32)


if __name__ == "__main__":
    rng = np.random.default_rng(0)
    A = rng.standard_normal((B_TOTAL, P, P), dtype=np.float32) * 0.1
    sig = np.einsum("bij,bkj->bik", A, A) + 0.1 * np.eye(P, dtype=np.float32)
    bet = rng.random((B_TOTAL, P), dtype=np.float32)
    bet /= bet.sum(-1, keepdims=True)
    wp = np.full((B_TOTAL, P), 1.0 / P, dtype=np.float32)
    out = kernel(
        sigma=sig,
        beta=bet,
        w_prev=wp,
        log_lambda_sparse=np.float32(-3.0),
        log_lambda_turnover=np.float32(-2.0),
    )
    lam_s = np.exp(np.float32(-3.0))
    lam_t = np.exp(np.float32(-2.0))
    n = 256
    w = np.full((n, P), 1.0 / P, dtype=np.float32)
    for _ in range(N_ITER):
        Sw = np.einsum("bij,bj->bi", sig[:n], w)
        g = 2 * Sw - bet[:n] + lam_s * np.sign(w) + 2 * lam_t * (w - wp[:n])
        w = w - STEP * g
        for _ in range(2):
            w = np.clip(w, 0, MAXW)
            w = w / (w.sum(-1, keepdims=True) + 1e-8)
    err = np.abs(out[:n] - w).max() / np.abs(w).max()
    print(out.shape, out.dtype, "absmax-rel vs numpy (256 rows):", err)
